# revision 11
# baseline (speedup 1.0000x reference)
"""Trainium2 Bass kernel for MetapathAggreLayer.

Computes, per node n:
    score[n, c] = sum_h hs[c, n, h] * v[c, h]        (c = 8 channels)
    att = softmax(score, axis=c)
    out[n, h]   = sum_c att[n, c] * hs[c, n, h]

Sharding: data-parallel over nodes across 8 NeuronCores (50000 nodes each).
meta_att_vec is replicated (pre-broadcast across partitions on the host).

On-chip layout: nodes on the partition axis (128/tile), (channel, hid) on the
free axis. Per 256-node macro-tile:
  - DVE: hs * v_bcast multiply, segmented reduce -> score[n, c], softmax sums,
         reciprocal, final cross-channel reduce
  - ACT: exp, and the 16 per-(group, channel) weighted scalings att[n,c]*hs_c
  - DMA via HWDGE (nc.sync)
"""

import sys

if "/opt/trn_rl_repo" not in sys.path:
    sys.path.insert(0, "/opt/trn_rl_repo")

import numpy as np

NCH = 8
NNODE = 400000
NHID = 128
NCORES = 8
NPC = NNODE // NCORES  # 50000 nodes per core
P = 128
NG = 2  # node groups (of 128) per macro-tile
FULL_ITERS = NPC // (NG * P)  # 195
TAIL = NPC - FULL_ITERS * NG * P  # 80

_cache = {}


def _build_program():
    import concourse.bacc as bacc
    import concourse.tile as tile
    import concourse.mybir as mybir

    dt = mybir.dt.float32
    AX = mybir.AxisListType.X
    AF = mybir.ActivationFunctionType

    nc = bacc.Bacc("TRN2", target_bir_lowering=False, debug=False)
    hs_d = nc.dram_tensor("hs", [NCH, NPC, NHID], dt, kind="ExternalInput").ap()
    vb_d = nc.dram_tensor("vb", [P, NCH, NHID], dt, kind="ExternalInput").ap()
    out_d = nc.dram_tensor("out", [NPC, NHID], dt, kind="ExternalOutput").ap()

    with tile.TileContext(nc) as tc:
        with (
            tc.tile_pool(name="const", bufs=1) as cpool,
            tc.tile_pool(name="hs", bufs=6) as hpool,
            tc.tile_pool(name="prod", bufs=5) as ppool,
            tc.tile_pool(name="wsum", bufs=6) as wpool,
            tc.tile_pool(name="small", bufs=12) as spool,
            tc.tile_pool(name="outp", bufs=8) as opool,
        ):
            vb = cpool.tile([P, NCH, NHID], dt)
            nc.sync.dma_start(vb[:], vb_d[:])

            def body(base, ng, p):
                n = ng * p
                hs_t = hpool.tile([P, ng, NCH, NHID], dt, tag="hs")
                for g in range(ng):
                    nc.sync.dma_start(
                        hs_t[:p, g],
                        hs_d[:, base + g * p : base + (g + 1) * p, :].rearrange(
                            "c p h -> p c h"
                        ),
                    )

                # score[n, (g c)] = sum_h hs * v
                prod = ppool.tile([P, ng, NCH, NHID], dt, tag="prod")
                vbb = vb[:p].unsqueeze(1).broadcast_to([p, ng, NCH, NHID])
                nc.vector.tensor_mul(prod[:p], hs_t[:p], vbb)
                score = spool.tile([P, ng, NCH], dt, tag="score")
                nc.vector.reduce_sum(score[:p], prod[:p], axis=AX)

                # softmax over c (8 wide); scores are O(10), exp is safe in fp32
                e = spool.tile([P, ng, NCH], dt, tag="e")
                s = spool.tile([P, ng], dt, tag="s")
                for g in range(ng):
                    nc.scalar.activation(
                        e[:p, g, :], score[:p, g, :], AF.Exp,
                        accum_out=s[:p, g : g + 1],
                    )
                r = spool.tile([P, ng], dt, tag="r")
                nc.vector.reciprocal(r[:p], s[:p])
                att = spool.tile([P, ng, NCH], dt, tag="att")
                for g in range(ng):
                    nc.vector.tensor_scalar_mul(
                        att[:p, g, :], e[:p, g, :], r[:p, g : g + 1]
                    )

                # weighted sum: wsum[n, g, h, c] = att[n,(g c)] * hs; scalings
                # mostly on ACT (2 of 16 on DVE), cross-channel reduce on DVE
                wsum = wpool.tile([P, ng, NHID, NCH], dt, tag="wsum")
                for g in range(ng):
                    for c in range(NCH):
                        nc.scalar.mul(
                            wsum[:p, g, :, c],
                            hs_t[:p, g, c, :],
                            att[:p, g, c : c + 1],
                        )
                out_t = opool.tile([P, ng, NHID], dt, tag="out")
                for g in range(ng):
                    nc.vector.reduce_sum(out_t[:p, g], wsum[:p, g], axis=AX)
                    nc.sync.dma_start(
                        out_d[base + g * p : base + (g + 1) * p, :].rearrange(
                            "(o p) h -> p o h", p=p
                        ),
                        out_t[:p, g : g + 1],
                    )

            for i in range(FULL_ITERS):
                body(i * NG * P, NG, P)
            if TAIL:
                body(FULL_ITERS * NG * P, 1, TAIL)

    nc.compile()
    return nc


def _get_program():
    if "nc" not in _cache:
        _cache["nc"] = _build_program()
    return _cache["nc"]


def run(hs, meta_att_vec, trace=False):
    from concourse.bass_utils import run_bass_kernel_spmd

    nc = _get_program()
    hs = np.asarray(hs, dtype=np.float32)
    v = np.asarray(meta_att_vec, dtype=np.float32)
    vb = np.ascontiguousarray(
        np.broadcast_to(v.reshape(1, NCH, NHID), (P, NCH, NHID))
    )
    in_maps = [
        {
            "hs": np.ascontiguousarray(hs[:, i * NPC : (i + 1) * NPC, :]),
            "vb": vb,
        }
        for i in range(NCORES)
    ]
    res = run_bass_kernel_spmd(nc, in_maps, list(range(NCORES)), trace=trace)
    out = np.concatenate([res.results[i]["out"] for i in range(NCORES)], axis=0)
    return out, res


def kernel(hs, meta_att_vec, nnode=None):
    out, _ = run(hs, meta_att_vec, trace=False)
    return out
